# revision 47
# baseline (speedup 1.0000x reference)
"""Trainium2 Bass kernel for nn_ClassifierI (12-layer GPT-2-style classifier).

Strategy: pure data-parallel over batch. B=16 sequences are split 2 per
NeuronCore across 8 cores; each core runs the full transformer on its
2x512 tokens with zero collectives, and the host gathers the [2,2] logits.

On-chip layout: activations are kept TRANSPOSED ([C on partitions, tokens on
free]) so every GEMM, the attention score/AV matmuls and all bias/LayerNorm
affine folds are native:
  - x (fp32 residual), xn (LN output, bf16), y (attn out, bf16): [128, 8, 1024]
  - q^T/k^T produced per head-pair, v in token-major with a ones-column
    appended per head so the AV matmul also produces the softmax denominator
  - scores are computed transposed (s^T[k, q] = k @ q^T) so softmax needs no
    transposes anywhere; the per-query normalizer is applied to the 64-row
    AV output via a DMA partition-broadcast of 1/sumexp
LayerNorm reduces over partitions via ones-matmuls in fp32r; rstd is
exp(-0.5*ln(var+eps)) so one ACT table set covers LN + softmax. All LN
affines and linear biases are exactly folded into weights / per-partition
bias vectors on the host.
"""

import math
import sys

import numpy as np

for _p in ("/opt/trn_rl_repo",):
    if _p not in sys.path:
        sys.path.insert(0, _p)

import ml_dtypes  # noqa: E402

B, T, C, H, L, V = 16, 512, 1024, 16, 12, 20
D = C // H              # 64
P = 128
NCORES = 8
BLOC = B // NCORES      # 2 sequences per core
NTOK = BLOC * T         # 1024 tokens per core
CT = C // P             # 8 C-tiles
FT = 4 * C // P         # 32 tiles of the 4C dim
KT = T // P             # 4 key tiles per sequence
NSEQ = BLOC             # 2
BF = ml_dtypes.bfloat16

_BUILT = {}


def _build(nl=L, dyn=True):
    import concourse.bass as bass
    import concourse.tile as tile
    from concourse import bacc, mybir
    from contextlib import ExitStack

    dt = mybir.dt
    f32, bf16, f32r = dt.float32, dt.bfloat16, dt.float32r
    AF = mybir.ActivationFunctionType
    ALU = mybir.AluOpType

    nc = bacc.Bacc("TRN2", target_bir_lowering=False, debug=False,
                   enable_asserts=False, num_devices=NCORES)

    def din(name, shape, dtype):
        return nc.dram_tensor(name, shape, dtype, kind="ExternalInput").ap()

    oh_d = din("oh", [32, NTOK], f32)
    wte_d = din("wte_p", [32, C], f32)
    wpeT_d = din("wpeT", [CT, P, T], f32)
    qkw_d = din("qkw", [nl * 16, P, 1024], bf16)
    vw_d = din("vw", [nl * 8, P, 1024], bf16)
    pw_d = din("pw", [nl * 8, P, 1024], bf16)
    fcw_d = din("fcw", [nl * 32, P, 1024], bf16)
    fpw_d = din("fpw", [nl * 8, P, 4096], bf16)
    qkb_d = din("qkb", [nl, P, 16], f32)
    drb_d = din("drb", [nl, P, 8], f32)
    fcb_d = din("fcb", [nl, P, 32], f32)
    r2b_d = din("r2b", [nl, P, 8], f32)
    mask_d = din("mask", [P, P], bf16)       # additive: -30000 above diagonal
    ident_d = din("ident", [P, P], bf16)
    onesk_d = din("onesk", [P, 1], f32r)
    onesr_d = din("onesr", [1, P], f32r)
    hw_d = din("hw", [P, CT * 2], bf16)
    hb_d = din("hb", [2, 1], f32)
    out_d = nc.dram_tensor("out", [2, NSEQ], f32, kind="ExternalOutput").ap()

    def wsel(ap, idx):
        # Select index idx (python int or loop-register ScalarValue) on dim 0.
        if isinstance(idx, int):
            return ap[idx]
        return ap[bass.ds(idx, 1)].rearrange("a p f -> (a p) f")

    with tile.TileContext(nc) as tc:
        with ExitStack() as ctx:
            ep = ctx.enter_context
            const = ep(tc.tile_pool(name="const", bufs=1))
            persist = ep(tc.tile_pool(name="persist", bufs=1))
            qkp = ep(tc.tile_pool(name="qkp", bufs=4))
            gpool = ep(tc.tile_pool(name="gpool", bufs=1))
            vwpool = ep(tc.tile_pool(name="vwpool", bufs=CT))
            wpool = ep(tc.tile_pool(name="wpool", bufs=3))
            fcwpool = ep(tc.tile_pool(name="fcwpool", bufs=6))
            w2pool = ep(tc.tile_pool(name="w2pool", bufs=3))
            scr = ep(tc.tile_pool(name="scr", bufs=5))
            rows = ep(tc.tile_pool(name="rows", bufs=4))
            lnp = ep(tc.tile_pool(name="lnp", bufs=1))
            epool = ep(tc.tile_pool(name="epool", bufs=4))
            bpool = ep(tc.tile_pool(name="bpool", bufs=2))
            dramp = ep(tc.tile_pool(name="dramp", bufs=6, space="DRAM"))
            ps_mm = ep(tc.tile_pool(name="ps_mm", bufs=2, space="PSUM"))
            ps_sc = ep(tc.tile_pool(name="ps_sc", bufs=3, space="PSUM"))
            ps_av = ep(tc.tile_pool(name="ps_av", bufs=3, space="PSUM"))

            # ---- persistent tensors ----
            # x is float32r: residual precision is fp32r (>= tf32), and the
            # LayerNorm stats matmuls can then read x directly at full rate.
            x = persist.tile([P, CT, NTOK], f32r, tag="x")
            xn = persist.tile([P, CT, NTOK], bf16, tag="xn")
            y = persist.tile([P, CT, NTOK], bf16, tag="y")
            vext = persist.tile([P, NSEQ * KT, H, D + 1], bf16, tag="vext")

            mask_sb = const.tile([P, P], bf16, tag="mask")
            nc.sync.dma_start(mask_sb[:], mask_d[:])
            ident_sb = const.tile([P, P], bf16, tag="ident")
            nc.sync.dma_start(ident_sb[:], ident_d[:])
            onesk = const.tile([P, 1], f32r, tag="onesk")
            nc.sync.dma_start(onesk[:], onesk_d[:])
            onesr = const.tile([1, P], f32r, tag="onesr")
            nc.sync.dma_start(onesr[:], onesr_d[:])
            hw_sb = const.tile([P, CT * 2], bf16, tag="hw")
            nc.sync.dma_start(hw_sb[:], hw_d[:])
            hb_sb = const.tile([2, 1], f32, tag="hb")
            nc.sync.dma_start(hb_sb[:], hb_d[:])
            eps11 = const.tile([1, 1], f32, tag="eps")
            nc.vector.memset(eps11[:], 1e-5)
            # ones column of vext (appended row of ones -> sumexp via AV matmul)
            nc.vector.memset(vext[:, :, :, D:D + 1], 1.0)

            # ---- embedding prologue: x = (onehot @ wte + wpe)^T ----
            for s in range(NSEQ):
                oh_sb = scr.tile([32, T], f32, tag="sc")
                nc.sync.dma_start(oh_sb[:32], oh_d[:, s * T:(s + 1) * T])
                for half in range(2):
                    wte_sb = scr.tile([32, T], f32, tag="sc")
                    nc.sync.dma_start(wte_sb[:32], wte_d[:, half * 512:(half + 1) * 512])
                    for ct4 in range(4):
                        ct = half * 4 + ct4
                        ps = ps_mm.tile([P, T], f32, tag="mm")
                        nc.tensor.matmul(ps[:], wte_sb[:32, ct4 * P:(ct4 + 1) * P],
                                         oh_sb[:32], start=True, stop=True)
                        wp = scr.tile([P, T], f32, tag="sc")
                        nc.sync.dma_start(wp[:], wpeT_d[ct])
                        nc.vector.tensor_add(x[:, ct, s * T:(s + 1) * T], ps[:], wp[:])

            # ---- layernorm over partitions: x -> dst (normalized, bf16) ----
            # Ln/Exp run once on a concatenated [1, NTOK] row gated on BOTH
            # chunks' stats, so ACT table switches can't interleave with the
            # gelu/exp phases (1.28us per table load otherwise).
            def layer_norm(dst):
                varcat = lnp.tile([1, NTOK], f32, tag="varcat")
                rstdcat = lnp.tile([1, NTOK], f32r, tag="rstdcat")
                stats = []
                for ch in range(NSEQ):
                    cs = slice(ch * T, (ch + 1) * T)
                    mps = ps_sc.tile([P, T], f32, tag="sc")
                    sqps = ps_sc.tile([P, T], f32, tag="sc")
                    for k in range(CT):
                        nc.tensor.matmul(mps[0:1, :], onesk[:], x[:, k, cs],
                                         start=(k == 0), stop=(k == CT - 1))
                        sq = scr.tile([P, T], f32r, tag="sc")
                        nc.scalar.activation(sq[:], x[:, k, cs], AF.Square)
                        nc.tensor.matmul(sqps[0:1, :], onesk[:], sq[:],
                                         start=(k == 0), stop=(k == CT - 1))
                    t1 = rows.tile([1, T], f32, tag="r")
                    nc.scalar.activation(t1[:], mps[0:1, :], AF.Square)
                    nc.vector.tensor_sub(varcat[:, cs], sqps[0:1, :], t1[:])
                    stats.append(mps)
                nc.scalar.activation(varcat[:], varcat[:], AF.Ln, bias=eps11[:])
                nc.scalar.activation(rstdcat[:], varcat[:], AF.Exp, scale=-0.5)
                for ch in range(NSEQ):
                    cs = slice(ch * T, (ch + 1) * T)
                    mr = rows.tile([1, T], f32r, tag="rf")
                    nc.vector.tensor_mul(mr[:], stats[ch][0:1, :], rstdcat[:, cs])
                    # broadcast rstd/mr to 128 partitions via K=1 matmuls
                    rb = ps_sc.tile([P, T], f32, tag="sc")
                    nc.tensor.matmul(rb[:], onesr[:], rstdcat[:, cs],
                                     start=True, stop=True)
                    mb = ps_sc.tile([P, T], f32, tag="sc")
                    nc.tensor.matmul(mb[:], onesr[:], mr[:], start=True, stop=True)
                    for k in range(CT):
                        tt = scr.tile([P, T], f32, tag="sc")
                        nc.vector.tensor_mul(tt[:], x[:, k, cs], rb[:])
                        nc.vector.tensor_sub(dst[:, k, cs], tt[:], mb[:])

            # ---- one transformer layer ----
            def emit_layer(lv):
                qkb = bpool.tile([P, 16], f32, tag="qkb")
                nc.sync.dma_start(qkb[:], wsel(qkb_d, lv))
                drb = bpool.tile([P, 8], f32, tag="drb")
                nc.sync.dma_start(drb[:], wsel(drb_d, lv))
                fcb = bpool.tile([P, 32], f32, tag="fcb")
                nc.sync.dma_start(fcb[:], wsel(fcb_d, lv))
                r2b = bpool.tile([P, 8], f32, tag="r2b")
                nc.sync.dma_start(r2b[:], wsel(r2b_d, lv))

                layer_norm(xn)

                # V gemm: v[tok, vcol] = (xn^T stationary) x Wv, into vext
                vw_tiles = []
                for k in range(CT):
                    vw_sb = vwpool.tile([P, 1024], bf16, tag="vw")
                    nc.sync.dma_start(vw_sb[:], wsel(vw_d, lv * 8 + k))
                    vw_tiles.append(vw_sb)
                for ti in range(CT):
                    for vc in range(2):
                        ps = ps_mm.tile([P, T], f32, tag="mm")
                        for k in range(CT):
                            nc.tensor.matmul(
                                ps[:], xn[:, k, ti * P:(ti + 1) * P],
                                vw_tiles[k][:, vc * 512:(vc + 1) * 512],
                                start=(k == 0), stop=(k == CT - 1))
                        nc.vector.tensor_copy(
                            vext[:, ti, vc * 8:(vc + 1) * 8, 0:D],
                            ps[:].rearrange("p (h d) -> p h d", h=8))

                # QK gemm interleaved with attention, head-pair major
                for j in range(CT):
                    qt = qkp.tile([P, NTOK], bf16, tag="qk")
                    kt = qkp.tile([P, NTOK], bf16, tag="qk")
                    for dst, m in ((qt, j), (kt, 8 + j)):
                        qkw_sb = wpool.tile([P, 1024], bf16, tag="qkw")
                        nc.sync.dma_start(qkw_sb[:], wsel(qkw_d, lv * 16 + m))
                        ps0 = ps_mm.tile([P, T], f32, tag="mm")
                        ps1 = ps_mm.tile([P, T], f32, tag="mm")
                        pss = [ps0, ps1]
                        for k in range(CT):
                            for n in range(2):
                                nc.tensor.matmul(
                                    pss[n][:], qkw_sb[:, k * P:(k + 1) * P],
                                    xn[:, k, n * T:(n + 1) * T],
                                    start=(k == 0), stop=(k == CT - 1))
                        for n in range(2):
                            nc.vector.tensor_scalar_add(dst[:, n * T:(n + 1) * T],
                                                        pss[n][:], qkb[:, m:m + 1])
                    for hh in range(2):
                        h = 2 * j + hh
                        hs = slice(hh * D, (hh + 1) * D)
                        for s in range(NSEQ):
                            av = ps_av.tile([D + 1, T], f32, tag="av")
                            for i in range(KT):
                                lo = i * P
                                sp = ps_sc.tile([P, T], f32, tag="sc")
                                nc.tensor.matmul(
                                    sp[:, lo:T],
                                    kt[hs, s * T + lo:s * T + lo + P],
                                    qt[hs, s * T + lo:(s + 1) * T],
                                    start=True, stop=False)
                                # additive causal mask folded in on the PE
                                nc.tensor.matmul(sp[:, lo:lo + P], ident_sb[:],
                                                 mask_sb[:], start=False, stop=True)
                                es = epool.tile([P, T], bf16, tag="es")
                                nc.scalar.activation(es[:, lo:T], sp[:, lo:T],
                                                     AF.Exp, scale=1.0 / math.sqrt(D))
                                nc.tensor.matmul(
                                    av[:, lo:T], vext[:, s * KT + i, h, :],
                                    es[:, lo:T],
                                    start=(i == 0), stop=(i == KT - 1))
                            rr = rows.tile([1, T], f32, tag="r")
                            nc.vector.reciprocal(rr[:], av[D:D + 1, :])
                            rd = dramp.tile([1, T], f32, tag="row")
                            nc.gpsimd.dma_start(rd[:], rr[:])
                            bc = scr.tile([P, T], f32, tag="sc")
                            nc.gpsimd.dma_start(bc[0:D, :], rd[:].to_broadcast((D, T)))
                            nc.vector.tensor_mul(
                                y[hs, j, s * T:(s + 1) * T], av[0:D, :], bc[0:D, :])

                # attn out projection + residual (+ folded bias)
                for m in range(CT):
                    pw_sb = wpool.tile([P, 1024], bf16, tag="pw")
                    nc.sync.dma_start(pw_sb[:], wsel(pw_d, lv * 8 + m))
                    ps0 = ps_mm.tile([P, T], f32, tag="mm")
                    ps1 = ps_mm.tile([P, T], f32, tag="mm")
                    pss = [ps0, ps1]
                    for k in range(CT):
                        for n in range(2):
                            nc.tensor.matmul(
                                pss[n][:], pw_sb[:, k * P:(k + 1) * P],
                                y[:, k, n * T:(n + 1) * T],
                                start=(k == 0), stop=(k == CT - 1))
                    for n in range(2):
                        cs = slice(n * T, (n + 1) * T)
                        nc.vector.scalar_tensor_tensor(
                            out=x[:, m, cs], in0=pss[n][:], scalar=drb[:, m:m + 1],
                            in1=x[:, m, cs], op0=ALU.add, op1=ALU.add)

                layer_norm(xn)

                # MLP (per 512-token chunk to halve the gelu buffer)
                for cch in range(NSEQ):
                    cs = slice(cch * T, (cch + 1) * T)
                    g = gpool.tile([P, FT, T], bf16, tag="g")
                    for m in range(FT):
                        fcw_sb = fcwpool.tile([P, 1024], bf16, tag="fcw")
                        nc.sync.dma_start(fcw_sb[:], wsel(fcw_d, lv * 32 + m))
                        ps = ps_mm.tile([P, T], f32, tag="mm")
                        for k in range(CT):
                            nc.tensor.matmul(
                                ps[:], fcw_sb[:, k * P:(k + 1) * P], xn[:, k, cs],
                                start=(k == 0), stop=(k == CT - 1))
                        nc.scalar.activation(g[:, m, :], ps[:], AF.Gelu_apprx_tanh,
                                             bias=fcb[:, m:m + 1])
                    for m in range(CT):
                        ps = ps_mm.tile([P, T], f32, tag="mm")
                        for kg in range(4):
                            fpw_sb = w2pool.tile([P, 1024], bf16, tag="fpw")
                            src = wsel(fpw_d, lv * 8 + m)
                            nc.sync.dma_start(fpw_sb[:],
                                              src[:, kg * 1024:(kg + 1) * 1024])
                            for k8 in range(8):
                                k = kg * 8 + k8
                                nc.tensor.matmul(
                                    ps[:], fpw_sb[:, k8 * P:(k8 + 1) * P], g[:, k, :],
                                    start=(k == 0), stop=(k == FT - 1))
                        nc.vector.scalar_tensor_tensor(
                            out=x[:, m, cs], in0=ps[:], scalar=r2b[:, m:m + 1],
                            in1=x[:, m, cs], op0=ALU.add, op1=ALU.add)

            if dyn:
                hint = (mybir.EngineType.PE, mybir.EngineType.DVE,
                        mybir.EngineType.Activation, mybir.EngineType.SP,
                        mybir.EngineType.Pool)
                with tc.For_i(0, nl, 1, hint_engines=hint) as lv:
                    emit_layer(lv)
            else:
                for lv in range(nl):
                    emit_layer(lv)

            # ---- final LN + classifier head ----
            layer_norm(xn)
            out_sb = const.tile([2, NSEQ], f32, tag="outsb")
            for s in range(NSEQ):
                ps = ps_mm.tile([2, T], f32, tag="mm")
                for k in range(CT):
                    nc.tensor.matmul(ps[:], hw_sb[:, k * 2:(k + 1) * 2],
                                     xn[:, k, s * T:(s + 1) * T],
                                     start=(k == 0), stop=(k == CT - 1))
                th = scr.tile([2, T], f32, tag="sc")
                nc.scalar.activation(th[:], ps[:], AF.Tanh, bias=hb_sb[:], scale=0.3)
                red = rows.tile([2, 1], f32, tag="red")
                nc.vector.tensor_reduce(red[:], th[:], mybir.AxisListType.X, ALU.add)
                nc.vector.tensor_scalar_mul(out_sb[:, s:s + 1], red[:], 3.0 / T)
            nc.sync.dma_start(out_d[:], out_sb[:])

    nc.compile()
    return nc


def _prep_host(inputs, nl=L):
    i = {k: np.asarray(v) for k, v in inputs.items()}
    f32 = np.float32

    ln1w, ln1b = i["ln1_w"].astype(f32), i["ln1_b"].astype(f32)
    ln2w, ln2b = i["ln2_w"].astype(f32), i["ln2_b"].astype(f32)
    aw = i["attn_w"].astype(f32) * ln1w[:, :, None]
    ab = np.einsum("lc,lcd->ld", ln1b, i["attn_w"].astype(f32)) + i["attn_b"].astype(f32)
    cv = ab[:, 2 * C:]
    dr = np.einsum("lc,lcd->ld", cv, i["proj_w"].astype(f32)) + i["proj_b"].astype(f32)
    fw = i["fc_w"].astype(f32) * ln2w[:, :, None]
    bfc = np.einsum("lc,lcd->ld", ln2b, i["fc_w"].astype(f32)) + i["fc_b"].astype(f32)
    r2 = i["fcproj_b"].astype(f32)
    hw = i["head_w"].astype(f32) * i["lnf_w"].astype(f32)[:, None]
    hb = i["lnf_b"].astype(f32) @ i["head_w"].astype(f32) + i["head_b"].astype(f32)

    qkw = aw[:, :, :2 * C].reshape(L, 8, P, 16, P).transpose(0, 3, 2, 1, 4) \
        .reshape(L * 16, P, 1024)[:nl * 16].astype(BF)
    vw = aw[:, :, 2 * C:].reshape(L, 8, P, 1024)[:nl].reshape(nl * 8, P, 1024).astype(BF)
    pw = i["proj_w"].astype(f32).reshape(L, 8, P, 8, P).transpose(0, 3, 2, 1, 4) \
        .reshape(L * 8, P, 1024)[:nl * 8].astype(BF)
    fcw = fw.reshape(L, 8, P, 32, P).transpose(0, 3, 2, 1, 4) \
        .reshape(L * 32, P, 1024)[:nl * 32].astype(BF)
    fpw = i["fcproj_w"].astype(f32).reshape(L, 32, P, 8, P).transpose(0, 3, 2, 1, 4) \
        .reshape(L * 8, P, 4096)[:nl * 8].astype(BF)
    qkb = np.ascontiguousarray(ab[:, :2 * C].reshape(L, 16, P).transpose(0, 2, 1))[:nl].astype(f32)
    drb = np.ascontiguousarray(dr.reshape(L, 8, P).transpose(0, 2, 1))[:nl].astype(f32)
    fcbv = np.ascontiguousarray(bfc.reshape(L, 32, P).transpose(0, 2, 1))[:nl].astype(f32)
    r2b = np.ascontiguousarray(r2.reshape(L, 8, P).transpose(0, 2, 1))[:nl].astype(f32)

    wte_p = np.zeros((32, C), f32)
    wte_p[:V] = i["wte"].astype(f32)
    wpeT = np.ascontiguousarray(i["wpe"].astype(f32).T).reshape(CT, P, T)
    # s^T[k_r, q_c] in a diagonal tile is masked (k > q) strictly below the
    # diagonal: add -30000 there so exp(scale*(s-30000)) underflows to 0.
    mask = (np.tril(np.full((P, P), -30000.0, f32), -1)).astype(BF)
    ident = np.eye(P, dtype=f32).astype(BF)
    hw_t = np.ascontiguousarray(hw.reshape(CT, P, 2).transpose(1, 0, 2)) \
        .reshape(P, CT * 2).astype(BF)
    hb_t = hb.reshape(2, 1).astype(f32)

    idx = i["idx"].astype(np.int64)
    shared = dict(wte_p=wte_p, wpeT=wpeT, qkw=qkw, vw=vw, pw=pw, fcw=fcw, fpw=fpw,
                  qkb=qkb, drb=drb, fcb=fcbv, r2b=r2b, mask=mask, hw=hw_t, hb=hb_t,
                  onesk=np.full((P, 1), 1.0 / C, np.float32),
                  onesr=np.ones((1, P), np.float32), ident=ident)
    in_maps = []
    for core in range(NCORES):
        seqs = idx[core * BLOC:(core + 1) * BLOC]          # [2, 512]
        oh = np.zeros((32, NTOK), f32)
        for s in range(BLOC):
            oh[seqs[s], np.arange(T) + s * T] = 1.0
        m = dict(shared)
        m["oh"] = oh
        in_maps.append(m)
    return in_maps


LAST_RESULTS = None


def kernel(**inputs):
    global LAST_RESULTS
    from concourse import bass_utils

    nl, dyn = L, True
    key = (nl, dyn)
    if key not in _BUILT:
        _BUILT[key] = _build(nl, dyn)
    nc = _BUILT[key]
    in_maps = _prep_host(inputs, nl)
    res = bass_utils.run_bass_kernel_spmd(nc, in_maps, core_ids=list(range(NCORES)))
    LAST_RESULTS = res
    out = np.zeros((B, 2), np.float32)
    for core in range(NCORES):
        o = res.results[core]["out"]                        # [2 classes, 2 seqs]
        out[core * BLOC:(core + 1) * BLOC] = o.T
    return out


# revision 48
# speedup vs baseline: 1.1063x; 1.1063x over previous
"""Trainium2 Bass kernel for nn_ClassifierI (12-layer GPT-2-style classifier).

Strategy: pure data-parallel over batch. B=16 sequences are split 2 per
NeuronCore across 8 cores; each core runs the full transformer on its
2x512 tokens with zero collectives, and the host gathers the [2,2] logits.

On-chip layout: activations are kept TRANSPOSED ([C on partitions, tokens on
free]) so every GEMM, the attention score/AV matmuls and all bias/LayerNorm
affine folds are native:
  - x (fp32 residual), xn (LN output, bf16), y (attn out, bf16): [128, 8, 1024]
  - q^T/k^T produced per head-pair, v in token-major with a ones-column
    appended per head so the AV matmul also produces the softmax denominator
  - scores are computed transposed (s^T[k, q] = k @ q^T) so softmax needs no
    transposes anywhere; the per-query normalizer is applied to the 64-row
    AV output via a DMA partition-broadcast of 1/sumexp
LayerNorm reduces over partitions via ones-matmuls in fp32r; rstd is
exp(-0.5*ln(var+eps)) so one ACT table set covers LN + softmax. All LN
affines and linear biases are exactly folded into weights / per-partition
bias vectors on the host.
"""

import math
import sys

import numpy as np

for _p in ("/opt/trn_rl_repo",):
    if _p not in sys.path:
        sys.path.insert(0, _p)

import ml_dtypes  # noqa: E402

B, T, C, H, L, V = 16, 512, 1024, 16, 12, 20
D = C // H              # 64
P = 128
NCORES = 8
BLOC = B // NCORES      # 2 sequences per core
NTOK = BLOC * T         # 1024 tokens per core
CT = C // P             # 8 C-tiles
FT = 4 * C // P         # 32 tiles of the 4C dim
KT = T // P             # 4 key tiles per sequence
NSEQ = BLOC             # 2
BF = ml_dtypes.bfloat16

_BUILT = {}


def _build(nl=L, dyn=True):
    import concourse.bass as bass
    import concourse.tile as tile
    from concourse import bacc, mybir
    from contextlib import ExitStack

    dt = mybir.dt
    f32, bf16, f32r = dt.float32, dt.bfloat16, dt.float32r
    AF = mybir.ActivationFunctionType
    ALU = mybir.AluOpType

    nc = bacc.Bacc("TRN2", target_bir_lowering=False, debug=False,
                   enable_asserts=False, num_devices=NCORES)

    def din(name, shape, dtype):
        return nc.dram_tensor(name, shape, dtype, kind="ExternalInput").ap()

    oh_d = din("oh", [32, NTOK], f32)
    wte_d = din("wte_p", [32, C], f32)
    wpeT_d = din("wpeT", [CT, P, T], f32)
    qkw_d = din("qkw", [nl * 16, P, 1024], bf16)
    vw_d = din("vw", [nl * 8, P, 1024], bf16)
    pw_d = din("pw", [nl * 8, P, 1024], bf16)
    fcw_d = din("fcw", [nl * 32, P, 1024], bf16)
    fpw_d = din("fpw", [nl * 8, P, 4096], bf16)
    qkb_d = din("qkb", [nl, P, 16], f32)
    drb_d = din("drb", [nl, P, 8], f32)
    fcb_d = din("fcb", [nl, P, 32], f32)
    r2b_d = din("r2b", [nl, P, 8], f32)
    mask_d = din("mask", [P, P], bf16)       # additive: -30000 above diagonal
    ident_d = din("ident", [P, P], bf16)
    onesk_d = din("onesk", [P, 1], f32r)
    onesr_d = din("onesr", [1, P], f32r)
    hw_d = din("hw", [P, CT * 2], bf16)
    hb_d = din("hb", [2, 1], f32)
    out_d = nc.dram_tensor("out", [2, NSEQ], f32, kind="ExternalOutput").ap()

    def wsel(ap, idx):
        # Select index idx (python int or loop-register ScalarValue) on dim 0.
        if isinstance(idx, int):
            return ap[idx]
        return ap[bass.ds(idx, 1)].rearrange("a p f -> (a p) f")

    with tile.TileContext(nc) as tc:
        with ExitStack() as ctx:
            ep = ctx.enter_context
            const = ep(tc.tile_pool(name="const", bufs=1))
            persist = ep(tc.tile_pool(name="persist", bufs=1))
            qkp = ep(tc.tile_pool(name="qkp", bufs=4))
            gpool = ep(tc.tile_pool(name="gpool", bufs=1))
            vwpool = ep(tc.tile_pool(name="vwpool", bufs=CT))
            wpool = ep(tc.tile_pool(name="wpool", bufs=3))
            fcwpool = ep(tc.tile_pool(name="fcwpool", bufs=6))
            w2pool = ep(tc.tile_pool(name="w2pool", bufs=3))
            scr = ep(tc.tile_pool(name="scr", bufs=5))
            rows = ep(tc.tile_pool(name="rows", bufs=4))
            lnp = ep(tc.tile_pool(name="lnp", bufs=1))
            epool = ep(tc.tile_pool(name="epool", bufs=4))
            bpool = ep(tc.tile_pool(name="bpool", bufs=2))
            dramp = ep(tc.tile_pool(name="dramp", bufs=6, space="DRAM"))
            ps_mm = ep(tc.tile_pool(name="ps_mm", bufs=2, space="PSUM"))
            ps_sc = ep(tc.tile_pool(name="ps_sc", bufs=3, space="PSUM"))
            ps_av = ep(tc.tile_pool(name="ps_av", bufs=3, space="PSUM"))

            # ---- persistent tensors ----
            # x is float32r: residual precision is fp32r (>= tf32), and the
            # LayerNorm stats matmuls can then read x directly at full rate.
            x = persist.tile([P, CT, NTOK], f32r, tag="x")
            xn = persist.tile([P, CT, NTOK], bf16, tag="xn")
            y = persist.tile([P, CT, NTOK], bf16, tag="y")
            vext = persist.tile([P, NSEQ * KT, H, D + 1], bf16, tag="vext")

            mask_sb = const.tile([P, P], bf16, tag="mask")
            nc.sync.dma_start(mask_sb[:], mask_d[:])
            ident_sb = const.tile([P, P], bf16, tag="ident")
            nc.sync.dma_start(ident_sb[:], ident_d[:])
            onesk = const.tile([P, 1], f32r, tag="onesk")
            nc.sync.dma_start(onesk[:], onesk_d[:])
            onesr = const.tile([1, P], f32r, tag="onesr")
            nc.sync.dma_start(onesr[:], onesr_d[:])
            hw_sb = const.tile([P, CT * 2], bf16, tag="hw")
            nc.sync.dma_start(hw_sb[:], hw_d[:])
            hb_sb = const.tile([2, 1], f32, tag="hb")
            nc.sync.dma_start(hb_sb[:], hb_d[:])
            eps11 = const.tile([1, 1], f32, tag="eps")
            nc.vector.memset(eps11[:], 1e-5)
            # ones column of vext (appended row of ones -> sumexp via AV matmul)
            nc.vector.memset(vext[:, :, :, D:D + 1], 1.0)

            # ---- embedding prologue: x = (onehot @ wte + wpe)^T ----
            for s in range(NSEQ):
                oh_sb = scr.tile([32, T], f32, tag="sc")
                nc.sync.dma_start(oh_sb[:32], oh_d[:, s * T:(s + 1) * T])
                for half in range(2):
                    wte_sb = scr.tile([32, T], f32, tag="sc")
                    nc.sync.dma_start(wte_sb[:32], wte_d[:, half * 512:(half + 1) * 512])
                    for ct4 in range(4):
                        ct = half * 4 + ct4
                        ps = ps_mm.tile([P, T], f32, tag="mm")
                        nc.tensor.matmul(ps[:], wte_sb[:32, ct4 * P:(ct4 + 1) * P],
                                         oh_sb[:32], start=True, stop=True)
                        wp = scr.tile([P, T], f32, tag="sc")
                        nc.sync.dma_start(wp[:], wpeT_d[ct])
                        nc.vector.tensor_add(x[:, ct, s * T:(s + 1) * T], ps[:], wp[:])

            # ---- layernorm over partitions: x -> dst (normalized, bf16) ----
            # Ln/Exp run once on a concatenated [1, NTOK] row gated on BOTH
            # chunks' stats, so ACT table switches can't interleave with the
            # gelu/exp phases (1.28us per table load otherwise).
            def layer_norm(dst):
                varcat = lnp.tile([1, NTOK], f32, tag="varcat")
                rstdcat = lnp.tile([1, NTOK], f32r, tag="rstdcat")
                stats = []
                for ch in range(NSEQ):
                    cs = slice(ch * T, (ch + 1) * T)
                    mps = ps_sc.tile([P, T], f32, tag="sc")
                    sqps = ps_sc.tile([P, T], f32, tag="sc")
                    for k in range(CT):
                        nc.tensor.matmul(mps[0:1, :], onesk[:], x[:, k, cs],
                                         start=(k == 0), stop=(k == CT - 1))
                        sq = scr.tile([P, T], f32r, tag="sc")
                        nc.scalar.activation(sq[:], x[:, k, cs], AF.Square)
                        nc.tensor.matmul(sqps[0:1, :], onesk[:], sq[:],
                                         start=(k == 0), stop=(k == CT - 1))
                    t1 = rows.tile([1, T], f32, tag="r")
                    nc.scalar.activation(t1[:], mps[0:1, :], AF.Square)
                    nc.vector.tensor_sub(varcat[:, cs], sqps[0:1, :], t1[:])
                    stats.append(mps)
                nc.scalar.activation(varcat[:], varcat[:], AF.Ln, bias=eps11[:])
                nc.scalar.activation(rstdcat[:], varcat[:], AF.Exp, scale=-0.5)
                for ch in range(NSEQ):
                    cs = slice(ch * T, (ch + 1) * T)
                    mr = rows.tile([1, T], f32r, tag="rf")
                    nc.vector.tensor_mul(mr[:], stats[ch][0:1, :], rstdcat[:, cs])
                    # broadcast rstd/mr to 128 partitions via K=1 matmuls
                    rb = ps_sc.tile([P, T], f32, tag="sc")
                    nc.tensor.matmul(rb[:], onesr[:], rstdcat[:, cs],
                                     start=True, stop=True)
                    mb = ps_sc.tile([P, T], f32, tag="sc")
                    nc.tensor.matmul(mb[:], onesr[:], mr[:], start=True, stop=True)
                    for k in range(CT):
                        tt = scr.tile([P, T], f32, tag="sc")
                        nc.vector.tensor_mul(tt[:], x[:, k, cs], rb[:])
                        nc.vector.tensor_sub(dst[:, k, cs], tt[:], mb[:])

            # ---- one transformer layer ----
            def emit_layer(lv):
                qkb = bpool.tile([P, 16], f32, tag="qkb")
                nc.sync.dma_start(qkb[:], wsel(qkb_d, lv))
                drb = bpool.tile([P, 8], f32, tag="drb")
                nc.sync.dma_start(drb[:], wsel(drb_d, lv))
                fcb = bpool.tile([P, 32], f32, tag="fcb")
                nc.sync.dma_start(fcb[:], wsel(fcb_d, lv))
                r2b = bpool.tile([P, 8], f32, tag="r2b")
                nc.sync.dma_start(r2b[:], wsel(r2b_d, lv))

                layer_norm(xn)

                # V gemm: v[tok, vcol] = (xn^T stationary) x Wv, into vext
                vw_tiles = []
                for k in range(CT):
                    vw_sb = vwpool.tile([P, 1024], bf16, tag="vw")
                    nc.sync.dma_start(vw_sb[:], wsel(vw_d, lv * 8 + k))
                    vw_tiles.append(vw_sb)
                for ti in range(CT):
                    for vc in range(2):
                        ps = ps_mm.tile([P, T], f32, tag="mm")
                        for k in range(CT):
                            nc.tensor.matmul(
                                ps[:], xn[:, k, ti * P:(ti + 1) * P],
                                vw_tiles[k][:, vc * 512:(vc + 1) * 512],
                                start=(k == 0), stop=(k == CT - 1))
                        nc.vector.tensor_copy(
                            vext[:, ti, vc * 8:(vc + 1) * 8, 0:D],
                            ps[:].rearrange("p (h d) -> p h d", h=8))

                # QK gemm interleaved with attention, head-pair major
                for j in range(CT):
                    qt = qkp.tile([P, NTOK], bf16, tag="qk")
                    kt = qkp.tile([P, NTOK], bf16, tag="qk")
                    for dst, m in ((qt, j), (kt, 8 + j)):
                        qkw_sb = wpool.tile([P, 1024], bf16, tag="qkw")
                        nc.sync.dma_start(qkw_sb[:], wsel(qkw_d, lv * 16 + m))
                        for n in range(2):
                            ps = ps_mm.tile([P, T], f32, tag="mm")
                            for k in range(CT):
                                nc.tensor.matmul(
                                    ps[:], qkw_sb[:, k * P:(k + 1) * P],
                                    xn[:, k, n * T:(n + 1) * T],
                                    start=(k == 0), stop=(k == CT - 1))
                            nc.vector.tensor_scalar_add(dst[:, n * T:(n + 1) * T],
                                                        ps[:], qkb[:, m:m + 1])
                    for hh in range(2):
                        h = 2 * j + hh
                        hs = slice(hh * D, (hh + 1) * D)
                        for s in range(NSEQ):
                            av = ps_av.tile([D + 1, T], f32, tag="av")
                            for i in range(KT):
                                lo = i * P
                                sp = ps_sc.tile([P, T], f32, tag="sc")
                                nc.tensor.matmul(
                                    sp[:, lo:T],
                                    kt[hs, s * T + lo:s * T + lo + P],
                                    qt[hs, s * T + lo:(s + 1) * T],
                                    start=True, stop=False)
                                # additive causal mask folded in on the PE
                                nc.tensor.matmul(sp[:, lo:lo + P], ident_sb[:],
                                                 mask_sb[:], start=False, stop=True)
                                es = epool.tile([P, T], bf16, tag="es")
                                nc.scalar.activation(es[:, lo:T], sp[:, lo:T],
                                                     AF.Exp, scale=1.0 / math.sqrt(D))
                                nc.tensor.matmul(
                                    av[:, lo:T], vext[:, s * KT + i, h, :],
                                    es[:, lo:T],
                                    start=(i == 0), stop=(i == KT - 1))
                            rr = rows.tile([1, T], f32, tag="r")
                            nc.vector.reciprocal(rr[:], av[D:D + 1, :])
                            rd = dramp.tile([1, T], f32, tag="row")
                            nc.gpsimd.dma_start(rd[:], rr[:])
                            bc = scr.tile([P, T], f32, tag="sc")
                            nc.gpsimd.dma_start(bc[0:D, :], rd[:].to_broadcast((D, T)))
                            nc.vector.tensor_mul(
                                y[hs, j, s * T:(s + 1) * T], av[0:D, :], bc[0:D, :])

                # attn out projection + residual (+ folded bias)
                for m in range(CT):
                    pw_sb = wpool.tile([P, 1024], bf16, tag="pw")
                    nc.sync.dma_start(pw_sb[:], wsel(pw_d, lv * 8 + m))
                    for n in range(2):
                        ps = ps_mm.tile([P, T], f32, tag="mm")
                        for k in range(CT):
                            nc.tensor.matmul(
                                ps[:], pw_sb[:, k * P:(k + 1) * P],
                                y[:, k, n * T:(n + 1) * T],
                                start=(k == 0), stop=(k == CT - 1))
                        cs = slice(n * T, (n + 1) * T)
                        nc.vector.scalar_tensor_tensor(
                            out=x[:, m, cs], in0=ps[:], scalar=drb[:, m:m + 1],
                            in1=x[:, m, cs], op0=ALU.add, op1=ALU.add)

                layer_norm(xn)

                # MLP (per 512-token chunk to halve the gelu buffer)
                for cch in range(NSEQ):
                    cs = slice(cch * T, (cch + 1) * T)
                    g = gpool.tile([P, FT, T], bf16, tag="g")
                    for m in range(FT):
                        fcw_sb = fcwpool.tile([P, 1024], bf16, tag="fcw")
                        nc.sync.dma_start(fcw_sb[:], wsel(fcw_d, lv * 32 + m))
                        ps = ps_mm.tile([P, T], f32, tag="mm")
                        for k in range(CT):
                            nc.tensor.matmul(
                                ps[:], fcw_sb[:, k * P:(k + 1) * P], xn[:, k, cs],
                                start=(k == 0), stop=(k == CT - 1))
                        nc.scalar.activation(g[:, m, :], ps[:], AF.Gelu_apprx_tanh,
                                             bias=fcb[:, m:m + 1])
                    for m in range(CT):
                        ps = ps_mm.tile([P, T], f32, tag="mm")
                        for kg in range(4):
                            fpw_sb = w2pool.tile([P, 1024], bf16, tag="fpw")
                            src = wsel(fpw_d, lv * 8 + m)
                            nc.sync.dma_start(fpw_sb[:],
                                              src[:, kg * 1024:(kg + 1) * 1024])
                            for k8 in range(8):
                                k = kg * 8 + k8
                                nc.tensor.matmul(
                                    ps[:], fpw_sb[:, k8 * P:(k8 + 1) * P], g[:, k, :],
                                    start=(k == 0), stop=(k == FT - 1))
                        nc.vector.scalar_tensor_tensor(
                            out=x[:, m, cs], in0=ps[:], scalar=r2b[:, m:m + 1],
                            in1=x[:, m, cs], op0=ALU.add, op1=ALU.add)

            if dyn:
                hint = (mybir.EngineType.PE, mybir.EngineType.DVE,
                        mybir.EngineType.Activation, mybir.EngineType.SP,
                        mybir.EngineType.Pool)
                with tc.For_i(0, nl, 1, hint_engines=hint) as lv:
                    emit_layer(lv)
            else:
                for lv in range(nl):
                    emit_layer(lv)

            # ---- final LN + classifier head ----
            layer_norm(xn)
            out_sb = const.tile([2, NSEQ], f32, tag="outsb")
            for s in range(NSEQ):
                ps = ps_mm.tile([2, T], f32, tag="mm")
                for k in range(CT):
                    nc.tensor.matmul(ps[:], hw_sb[:, k * 2:(k + 1) * 2],
                                     xn[:, k, s * T:(s + 1) * T],
                                     start=(k == 0), stop=(k == CT - 1))
                th = scr.tile([2, T], f32, tag="sc")
                nc.scalar.activation(th[:], ps[:], AF.Tanh, bias=hb_sb[:], scale=0.3)
                red = rows.tile([2, 1], f32, tag="red")
                nc.vector.tensor_reduce(red[:], th[:], mybir.AxisListType.X, ALU.add)
                nc.vector.tensor_scalar_mul(out_sb[:, s:s + 1], red[:], 3.0 / T)
            nc.sync.dma_start(out_d[:], out_sb[:])

    nc.compile()
    return nc


def _prep_host(inputs, nl=L):
    i = {k: np.asarray(v) for k, v in inputs.items()}
    f32 = np.float32

    ln1w, ln1b = i["ln1_w"].astype(f32), i["ln1_b"].astype(f32)
    ln2w, ln2b = i["ln2_w"].astype(f32), i["ln2_b"].astype(f32)
    aw = i["attn_w"].astype(f32) * ln1w[:, :, None]
    ab = np.einsum("lc,lcd->ld", ln1b, i["attn_w"].astype(f32)) + i["attn_b"].astype(f32)
    cv = ab[:, 2 * C:]
    dr = np.einsum("lc,lcd->ld", cv, i["proj_w"].astype(f32)) + i["proj_b"].astype(f32)
    fw = i["fc_w"].astype(f32) * ln2w[:, :, None]
    bfc = np.einsum("lc,lcd->ld", ln2b, i["fc_w"].astype(f32)) + i["fc_b"].astype(f32)
    r2 = i["fcproj_b"].astype(f32)
    hw = i["head_w"].astype(f32) * i["lnf_w"].astype(f32)[:, None]
    hb = i["lnf_b"].astype(f32) @ i["head_w"].astype(f32) + i["head_b"].astype(f32)

    qkw = aw[:, :, :2 * C].reshape(L, 8, P, 16, P).transpose(0, 3, 2, 1, 4) \
        .reshape(L * 16, P, 1024)[:nl * 16].astype(BF)
    vw = aw[:, :, 2 * C:].reshape(L, 8, P, 1024)[:nl].reshape(nl * 8, P, 1024).astype(BF)
    pw = i["proj_w"].astype(f32).reshape(L, 8, P, 8, P).transpose(0, 3, 2, 1, 4) \
        .reshape(L * 8, P, 1024)[:nl * 8].astype(BF)
    fcw = fw.reshape(L, 8, P, 32, P).transpose(0, 3, 2, 1, 4) \
        .reshape(L * 32, P, 1024)[:nl * 32].astype(BF)
    fpw = i["fcproj_w"].astype(f32).reshape(L, 32, P, 8, P).transpose(0, 3, 2, 1, 4) \
        .reshape(L * 8, P, 4096)[:nl * 8].astype(BF)
    qkb = np.ascontiguousarray(ab[:, :2 * C].reshape(L, 16, P).transpose(0, 2, 1))[:nl].astype(f32)
    drb = np.ascontiguousarray(dr.reshape(L, 8, P).transpose(0, 2, 1))[:nl].astype(f32)
    fcbv = np.ascontiguousarray(bfc.reshape(L, 32, P).transpose(0, 2, 1))[:nl].astype(f32)
    r2b = np.ascontiguousarray(r2.reshape(L, 8, P).transpose(0, 2, 1))[:nl].astype(f32)

    wte_p = np.zeros((32, C), f32)
    wte_p[:V] = i["wte"].astype(f32)
    wpeT = np.ascontiguousarray(i["wpe"].astype(f32).T).reshape(CT, P, T)
    # s^T[k_r, q_c] in a diagonal tile is masked (k > q) strictly below the
    # diagonal: add -30000 there so exp(scale*(s-30000)) underflows to 0.
    mask = (np.tril(np.full((P, P), -30000.0, f32), -1)).astype(BF)
    ident = np.eye(P, dtype=f32).astype(BF)
    hw_t = np.ascontiguousarray(hw.reshape(CT, P, 2).transpose(1, 0, 2)) \
        .reshape(P, CT * 2).astype(BF)
    hb_t = hb.reshape(2, 1).astype(f32)

    idx = i["idx"].astype(np.int64)
    shared = dict(wte_p=wte_p, wpeT=wpeT, qkw=qkw, vw=vw, pw=pw, fcw=fcw, fpw=fpw,
                  qkb=qkb, drb=drb, fcb=fcbv, r2b=r2b, mask=mask, hw=hw_t, hb=hb_t,
                  onesk=np.full((P, 1), 1.0 / C, np.float32),
                  onesr=np.ones((1, P), np.float32), ident=ident)
    in_maps = []
    for core in range(NCORES):
        seqs = idx[core * BLOC:(core + 1) * BLOC]          # [2, 512]
        oh = np.zeros((32, NTOK), f32)
        for s in range(BLOC):
            oh[seqs[s], np.arange(T) + s * T] = 1.0
        m = dict(shared)
        m["oh"] = oh
        in_maps.append(m)
    return in_maps


LAST_RESULTS = None


def kernel(**inputs):
    global LAST_RESULTS
    from concourse import bass_utils

    nl, dyn = L, True
    key = (nl, dyn)
    if key not in _BUILT:
        _BUILT[key] = _build(nl, dyn)
    nc = _BUILT[key]
    in_maps = _prep_host(inputs, nl)
    res = bass_utils.run_bass_kernel_spmd(nc, in_maps, core_ids=list(range(NCORES)))
    LAST_RESULTS = res
    out = np.zeros((B, 2), np.float32)
    for core in range(NCORES):
        o = res.results[core]["out"]                        # [2 classes, 2 seqs]
        out[core * BLOC:(core + 1) * BLOC] = o.T
    return out
